# revision 1
# baseline (speedup 1.0000x reference)
"""BitNet dense layer on 8 Trainium2 NeuronCores.

reference math:
    row_scale = clip(mean(|W|, axis=1), 1e-8)        # [out]
    out = (x @ sign(W).T) * row_scale * scale_param  # [B,S,out]

Strategy (data-parallel over the 8192 tokens):
  * Host folds row_scale * scale_param into the binarized weight:
        Wf = sign(W) * comb[:, None]   -> bf16, exactly +-comb[o] per row
    so the device kernel is a single plain matmul.
  * Host pre-transposes both operands so the device streams natural-layout
    [K, *] tiles (contraction dim on partitions) with zero on-chip transposes:
        xT [4096, 8192] bf16 (sharded 1024 tokens/core), wT [4096, 4096] bf16.
  * Each core computes out_c[1024, 4096] f32 = xT_c.T @ wT via the production
    tile matmul kernel; host concatenates the 8 shards.
"""

import os

import numpy as np
import ml_dtypes

B, S, D_IN, D_OUT = 4, 2048, 4096, 4096
N_CORES = 8
M_TOT = B * S
M_LOC = M_TOT // N_CORES

_prog = None
last_results = None  # BassKernelResults of the most recent run (for test harness)


def _build_program():
    import concourse.tile as tile
    from concourse import bacc, mybir
    from concourse.kernels.tile_matmul import matmul_tile_kernel

    nc = bacc.Bacc(
        "TRN2", target_bir_lowering=False, debug=False, num_devices=N_CORES
    )
    xT = nc.dram_tensor(
        "xT", [D_IN, M_LOC], mybir.dt.bfloat16, kind="ExternalInput"
    ).ap()
    wT = nc.dram_tensor(
        "wT", [D_IN, D_OUT], mybir.dt.bfloat16, kind="ExternalInput"
    ).ap()
    out = nc.dram_tensor(
        "out", [M_LOC, D_OUT], mybir.dt.float32, kind="ExternalOutput"
    ).ap()
    with tile.TileContext(nc) as tc:
        matmul_tile_kernel(
            tc,
            kxm_ap=xT,
            kxn_ap=wT,
            mxn_ap=out,
            MAX_K_TILE_SIZE=2048,
        )
    nc.compile()
    return nc


def kernel(input, weight, scale_param):
    global _prog, last_results
    from concourse.bass_utils import run_bass_kernel_spmd

    x = np.asarray(input, dtype=np.float32).reshape(M_TOT, D_IN)
    W = np.asarray(weight, dtype=np.float32)
    sp = np.asarray(scale_param, dtype=np.float32)

    comb = np.clip(np.abs(W).mean(axis=1, dtype=np.float32), 1e-8, None) * sp
    wT = (np.sign(W) * comb[:, None].astype(np.float32)).T.astype(
        ml_dtypes.bfloat16, order="C"
    )
    xT = x.T.astype(ml_dtypes.bfloat16, order="C")

    if _prog is None:
        _prog = _build_program()

    in_maps = [
        {
            "xT": np.ascontiguousarray(xT[:, c * M_LOC : (c + 1) * M_LOC]),
            "wT": wT,
        }
        for c in range(N_CORES)
    ]
    trace = bool(int(os.environ.get("BITNET_TRACE", "0")))
    last_results = run_bass_kernel_spmd(
        _prog, in_maps, list(range(N_CORES)), trace=trace
    )
    out = np.concatenate(
        [last_results.results[c]["out"] for c in range(N_CORES)], axis=0
    )
    return np.nan_to_num(
        out.reshape(B, S, D_OUT), nan=0.0, posinf=1e6, neginf=-1e6
    )


# revision 2
# speedup vs baseline: 1.0127x; 1.0127x over previous
"""BitNet dense layer on 8 Trainium2 NeuronCores.

reference math:
    row_scale = clip(mean(|W|, axis=1), 1e-8)        # [out]
    out = (x @ sign(W).T) * row_scale * scale_param  # [B,S,out]

Strategy (data-parallel over the 8192 tokens):
  * Host folds row_scale * scale_param into the binarized weight:
        Wf = sign(W) * comb[:, None]   -> bf16, exactly +-comb[o] per row
    so the device kernel is a single plain matmul.
  * Host pre-transposes both operands so the device streams natural-layout
    [K, *] tiles (contraction dim on partitions) with zero on-chip transposes:
        xT [4096, 8192] bf16 (sharded 1024 tokens/core), wT [4096, 4096] bf16.
  * Each core computes out_c[1024, 4096] f32 = xT_c.T @ wT via the production
    tile matmul kernel; host concatenates the 8 shards.
"""

import os

import numpy as np
import ml_dtypes

B, S, D_IN, D_OUT = 4, 2048, 4096, 4096
N_CORES = 8
M_TOT = B * S
M_LOC = M_TOT // N_CORES

_prog = None
last_results = None  # BassKernelResults of the most recent run (for test harness)


def _build_program():
    import concourse.tile as tile
    from concourse import bacc, mybir
    from concourse.kernels.tile_matmul import matmul_tile_kernel

    nc = bacc.Bacc(
        "TRN2", target_bir_lowering=False, debug=False, num_devices=N_CORES
    )
    xT = nc.dram_tensor(
        "xT", [D_IN, M_LOC], mybir.dt.bfloat16, kind="ExternalInput"
    ).ap()
    wT = nc.dram_tensor(
        "wT", [D_IN, D_OUT], mybir.dt.bfloat16, kind="ExternalInput"
    ).ap()
    out = nc.dram_tensor(
        "out", [M_LOC, D_OUT], mybir.dt.float32, kind="ExternalOutput"
    ).ap()
    with tile.TileContext(nc) as tc:
        matmul_tile_kernel(
            tc,
            kxm_ap=xT,
            kxn_ap=wT,
            mxn_ap=out,
        )
    nc.compile()
    return nc


def kernel(input, weight, scale_param):
    global _prog, last_results
    from concourse.bass_utils import run_bass_kernel_spmd

    x = np.asarray(input, dtype=np.float32).reshape(M_TOT, D_IN)
    W = np.asarray(weight, dtype=np.float32)
    sp = np.asarray(scale_param, dtype=np.float32)

    comb = np.clip(np.abs(W).mean(axis=1, dtype=np.float32), 1e-8, None) * sp
    wT = (np.sign(W) * comb[:, None].astype(np.float32)).T.astype(
        ml_dtypes.bfloat16, order="C"
    )
    xT = x.T.astype(ml_dtypes.bfloat16, order="C")

    if _prog is None:
        _prog = _build_program()

    in_maps = [
        {
            "xT": np.ascontiguousarray(xT[:, c * M_LOC : (c + 1) * M_LOC]),
            "wT": wT,
        }
        for c in range(N_CORES)
    ]
    trace = bool(int(os.environ.get("BITNET_TRACE", "0")))
    last_results = run_bass_kernel_spmd(
        _prog, in_maps, list(range(N_CORES)), trace=trace
    )
    out = np.concatenate(
        [last_results.results[c]["out"] for c in range(N_CORES)], axis=0
    )
    return np.nan_to_num(
        out.reshape(B, S, D_OUT), nan=0.0, posinf=1e6, neginf=-1e6
    )


# revision 4
# speedup vs baseline: 1.0200x; 1.0071x over previous
"""BitNet dense layer on 8 Trainium2 NeuronCores.

reference math:
    row_scale = clip(mean(|W|, axis=1), 1e-8)        # [out]
    out = (x @ sign(W).T) * row_scale * scale_param  # [B,S,out]

Strategy (data-parallel over the 8192 tokens):
  * Host folds row_scale * scale_param into the binarized weight:
        Wf = sign(W) * comb[:, None]   -> bf16, exactly +-comb[o] per row
    so the device kernel is a single plain matmul.
  * Host pre-transposes both operands so the device streams natural-layout
    [K, *] tiles (contraction dim on partitions) with zero on-chip transposes:
        xT [4096, 8192] bf16 (sharded 1024 tokens/core), wT [4096, 4096] bf16.
  * Each core computes out_c[1024, 4096] f32 = xT_c.T @ wT via the production
    tile matmul kernel; host concatenates the 8 shards.
"""

import os

import numpy as np
import ml_dtypes

B, S, D_IN, D_OUT = 4, 2048, 4096, 4096
N_CORES = 8
M_TOT = B * S
M_LOC = M_TOT // N_CORES

_prog = None
last_results = None  # BassKernelResults of the most recent run (for test harness)


def _build_program():
    import concourse.tile as tile
    from concourse import bacc, mybir
    from concourse.kernels.tile_matmul import matmul_tile_kernel

    nc = bacc.Bacc(
        "TRN2", target_bir_lowering=False, debug=False, num_devices=N_CORES
    )
    xT = nc.dram_tensor(
        "xT", [D_IN, M_LOC], mybir.dt.bfloat16, kind="ExternalInput"
    ).ap()
    wT = nc.dram_tensor(
        "wT", [D_IN, D_OUT], mybir.dt.bfloat16, kind="ExternalInput"
    ).ap()
    out = nc.dram_tensor(
        "out", [M_LOC, D_OUT], mybir.dt.float32, kind="ExternalOutput"
    ).ap()
    with tile.TileContext(nc) as tc:
        matmul_tile_kernel(
            tc,
            kxm_ap=xT,
            kxn_ap=wT,
            mxn_ap=out,
            MAX_K_TILE_SIZE=256,
        )
    nc.compile()
    return nc


def kernel(input, weight, scale_param):
    global _prog, last_results
    from concourse.bass_utils import run_bass_kernel_spmd

    x = np.asarray(input, dtype=np.float32).reshape(M_TOT, D_IN)
    W = np.asarray(weight, dtype=np.float32)
    sp = np.asarray(scale_param, dtype=np.float32)

    comb = np.clip(np.abs(W).mean(axis=1, dtype=np.float32), 1e-8, None) * sp
    wT = (np.sign(W) * comb[:, None].astype(np.float32)).T.astype(
        ml_dtypes.bfloat16, order="C"
    )
    xT = x.T.astype(ml_dtypes.bfloat16, order="C")

    if _prog is None:
        _prog = _build_program()

    in_maps = [
        {
            "xT": np.ascontiguousarray(xT[:, c * M_LOC : (c + 1) * M_LOC]),
            "wT": wT,
        }
        for c in range(N_CORES)
    ]
    trace = bool(int(os.environ.get("BITNET_TRACE", "0")))
    last_results = run_bass_kernel_spmd(
        _prog, in_maps, list(range(N_CORES)), trace=trace
    )
    out = np.concatenate(
        [last_results.results[c]["out"] for c in range(N_CORES)], axis=0
    )
    return np.nan_to_num(
        out.reshape(B, S, D_OUT), nan=0.0, posinf=1e6, neginf=-1e6
    )
